# revision 26
# baseline (speedup 1.0000x reference)
import os
import sys

import numpy as np

if "/opt/trn_rl_repo" not in sys.path:
    sys.path.insert(0, "/opt/trn_rl_repo")

import concourse.bass as bass
import concourse.mybir as mybir
import concourse.tile as tile
from concourse import bacc
from concourse.bass_utils import run_bass_kernel_spmd

P = 128
B, N, E = 64, 10000, 320000
LAMBDA_PHY = 0.3
NCORES = 8
EPC = E // NCORES            # 40000 real edges per core
G = (EPC + P - 1) // P       # 313 slot groups per core
EPAD = G * P                 # 40064 (64 pad entries per core)
CHUNK_SLOTS = 8              # 1024 idxs/gather: HW limit is <=1024 per dma_gather
NCHUNKS = (G + CHUNK_SLOTS - 1) // CHUNK_SLOTS  # 39 full chunks + 1-slot tail
NDL = N // NCORES            # 1250 data-loss columns per core
DL_F = B * NDL // P          # 625 free-dim elems for [128, 625] reshape

FP = mybir.dt.float32
I16 = mybir.dt.int16

LAST_EXEC_NS = None
LAST_PROFILE = None

_NC_CACHE = {}


def _build_nc():
    if "nc" in _NC_CACHE:
        return _NC_CACHE["nc"]
    nc = bacc.Bacc(None, target_bir_lowering=False, num_swdge_queues=4)

    table_d = nc.declare_dram_parameter("table", [N, P], FP, isOutput=False)
    sidx_d = nc.declare_dram_parameter("sidx", [P, EPAD // 16], I16, isOutput=False)
    didx_d = nc.declare_dram_parameter("didx", [P, EPAD // 16], I16, isOutput=False)
    c0_d = nc.declare_dram_parameter("c0a", [P, G], FP, isOutput=False)
    c1_d = nc.declare_dram_parameter("c1a", [P, G], FP, isOutput=False)
    c2_d = nc.declare_dram_parameter("c2a", [P, G], FP, isOutput=False)
    pdl_d = nc.declare_dram_parameter("pdl", [P, DL_F], FP, isOutput=False)
    tdl_d = nc.declare_dram_parameter("tdl", [P, DL_F], FP, isOutput=False)
    out_d = nc.declare_dram_parameter("partials", [P, 2], FP, isOutput=True)

    with tile.TileContext(nc) as tc:
        with tc.tile_pool(name="sbuf", bufs=1) as pool:
            sidx_t = pool.tile([P, EPAD // 16], I16)
            didx_t = pool.tile([P, EPAD // 16], I16)
            c0_t = pool.tile([P, G], FP)
            c1_t = pool.tile([P, G], FP)
            c2_t = pool.tile([P, G], FP)
            pdl_t = pool.tile([P, DL_F], FP)
            tdl_t = pool.tile([P, DL_F], FP)
            dd_t = pool.tile([P, DL_F], FP)
            sq_dl = pool.tile([P, DL_F], FP)
            phy_acc = pool.tile([P, 1], FP)
            dacc = pool.tile([P, 1], FP)
            chunk_accs = pool.tile([P, NCHUNKS], FP)

            NBUF = 4
            gs_t = [
                pool.tile([P, CHUNK_SLOTS, P], FP, name=f"gs{i}") for i in range(NBUF)
            ]
            gd_t = [
                pool.tile([P, CHUNK_SLOTS, P], FP, name=f"gd{i}") for i in range(NBUF)
            ]
            a0_t = pool.tile([P, CHUNK_SLOTS, B], FP)
            a1_t = pool.tile([P, CHUNK_SLOTS, B], FP)
            b_t = pool.tile([P, CHUNK_SLOTS, B], FP)
            c_t = pool.tile([P, CHUNK_SLOTS, B], FP)
            r_t = pool.tile([P, CHUNK_SLOTS, B], FP)

            nc.sync.dma_start(out=sidx_t[:], in_=sidx_d[:])
            nc.sync.dma_start(out=didx_t[:], in_=didx_d[:])
            nc.sync.dma_start(out=c0_t[:], in_=c0_d[:])
            nc.sync.dma_start(out=c1_t[:], in_=c1_d[:])
            nc.sync.dma_start(out=c2_t[:], in_=c2_d[:])
            nc.sync.dma_start(out=pdl_t[:], in_=pdl_d[:])
            nc.sync.dma_start(out=tdl_t[:], in_=tdl_d[:])

            # data loss partial: sum((pred - target)^2) over this core's shard
            # (tensor_tensor_reduce crashes the device on this toolchain, so
            # square + separate tensor_reduce instead)
            nc.vector.tensor_tensor(
                out=dd_t[:], in0=pdl_t[:], in1=tdl_t[:], op=mybir.AluOpType.subtract
            )
            nc.vector.tensor_tensor(
                out=sq_dl[:], in0=dd_t[:], in1=dd_t[:], op=mybir.AluOpType.mult
            )
            nc.vector.tensor_reduce(
                out=dacc[:],
                in_=sq_dl[:],
                axis=mybir.AxisListType.X,
                op=mybir.AluOpType.add,
            )

            GH = 4  # gather half: 512 idxs -> 33+33 ring descs, two coexist
            for k in range(NCHUNKS):
                so = k * CHUNK_SLOTS
                S = min(CHUNK_SLOTS, G - so)
                gs = gs_t[k % NBUF]
                gd = gd_t[k % NBUF]

                for hs in range(0, S, GH):
                    hS = min(GH, S - hs)
                    n_idx = hS * P
                    n_real = max(0, min(EPC - (so + hs) * P, n_idx))
                    col0 = (so + hs) * 8  # slot*128/16
                    ncol = n_idx // 16

                    if n_real < n_idx:
                        # pad entries (negative idxs) are skipped by the
                        # gather; zero their slots so the residual comes out 0
                        pad_p0 = n_real - (hS - 1) * P
                        sl = hs + hS - 1
                        nc.vector.memset(gs[pad_p0:P, sl : sl + 1, :], 0.0)
                        nc.vector.memset(gd[pad_p0:P, sl : sl + 1, :], 0.0)

                    nc.gpsimd.dma_gather(
                        out_ap=gs[:, hs : hs + hS, :],
                        in_ap=table_d[:, :],
                        idxs_ap=sidx_t[:, col0 : col0 + ncol],
                        num_idxs=n_idx,
                        num_idxs_reg=n_real,
                        elem_size=P,
                        queue_num=(2 * k) % 4,
                    )
                    nc.gpsimd.dma_gather(
                        out_ap=gd[:, hs : hs + hS, :],
                        in_ap=table_d[:, :],
                        idxs_ap=didx_t[:, col0 : col0 + ncol],
                        num_idxs=n_idx,
                        num_idxs_reg=n_real,
                        elem_size=P,
                        queue_num=(2 * k + 1) % 4,
                    )

                c0b = c0_t[:, so : so + S, None].to_broadcast([P, S, B])
                c1b = c1_t[:, so : so + S, None].to_broadcast([P, S, B])
                c2b = c2_t[:, so : so + S, None].to_broadcast([P, S, B])

                mul = mybir.AluOpType.mult
                sub = mybir.AluOpType.subtract
                # a0 = c0 * pred[src], a1 = c1 * prev[src], b = c2 * prev[dst]
                nc.vector.tensor_tensor(
                    out=a0_t[:, 0:S, :], in0=gs[:, 0:S, 0:B], in1=c0b, op=mul
                )
                nc.vector.tensor_tensor(
                    out=a1_t[:, 0:S, :], in0=gs[:, 0:S, B:P], in1=c1b, op=mul
                )
                nc.vector.tensor_tensor(
                    out=b_t[:, 0:S, :], in0=gd[:, 0:S, B:P], in1=c2b, op=mul
                )
                # r = pred[dst] - b - a0 - a1
                nc.vector.tensor_tensor(
                    out=c_t[:, 0:S, :], in0=gd[:, 0:S, 0:B], in1=b_t[:, 0:S, :], op=sub
                )
                nc.vector.tensor_tensor(
                    out=c_t[:, 0:S, :], in0=c_t[:, 0:S, :], in1=a0_t[:, 0:S, :], op=sub
                )
                nc.vector.tensor_tensor(
                    out=r_t[:, 0:S, :], in0=c_t[:, 0:S, :], in1=a1_t[:, 0:S, :], op=sub
                )
                # chunk_accs[:, k] = sum over (S, B) of r^2 per partition
                nc.vector.tensor_tensor(
                    out=b_t[:, 0:S, :], in0=r_t[:, 0:S, :], in1=r_t[:, 0:S, :], op=mul
                )
                nc.vector.tensor_reduce(
                    out=chunk_accs[:, k : k + 1],
                    in_=b_t[:, 0:S, :],
                    axis=mybir.AxisListType.XY,
                    op=mybir.AluOpType.add,
                )

            nc.vector.tensor_reduce(
                out=phy_acc[:],
                in_=chunk_accs[:],
                axis=mybir.AxisListType.X,
                op=mybir.AluOpType.add,
            )
            nc.sync.dma_start(out=out_d[:, 0:1], in_=phy_acc[:])
            nc.sync.dma_start(out=out_d[:, 1:2], in_=dacc[:])

    # Bacc.finalize runs the full lowering pipeline: wait splitting,
    # library loads for DMAGatherAnt, codegen_inst_isa_subclasses
    nc.finalize()
    _NC_CACHE["nc"] = nc
    return nc


def _wrap_idx(idx_pad: np.ndarray) -> np.ndarray:
    # dma_gather layout: index i lives at partition i%16, column i//16,
    # replicated across the 8 groups of 16 partitions
    w16 = idx_pad.reshape(EPAD // 16, 16).T  # [16, EPAD//16]
    return np.ascontiguousarray(np.tile(w16, (8, 1)))  # [128, EPAD//16]


def _arrange_coeff(c_shard: np.ndarray) -> np.ndarray:
    cp = np.zeros(EPAD, np.float32)
    cp[:EPC] = c_shard
    return np.ascontiguousarray(cp.reshape(G, P).T)  # [128, G]


def kernel(**inputs) -> np.ndarray:
    global LAST_EXEC_NS, LAST_PROFILE
    pred = np.ascontiguousarray(np.asarray(inputs["pred"], dtype=np.float32))
    target = np.ascontiguousarray(np.asarray(inputs["target"], dtype=np.float32))
    prev_target = np.ascontiguousarray(
        np.asarray(inputs["prev_target"], dtype=np.float32)
    )
    c0 = np.asarray(inputs["c0"], dtype=np.float32)
    c1 = np.asarray(inputs["c1"], dtype=np.float32)
    c2 = np.asarray(inputs["c2"], dtype=np.float32)
    edge_index = np.asarray(inputs["edge_index"])
    src = edge_index[0].astype(np.int16)
    dst = edge_index[1].astype(np.int16)

    # gather table: row n = [pred[:, n] | prev_target[:, n]]  (512B rows)
    table = np.ascontiguousarray(
        np.concatenate([pred.T, prev_target.T], axis=1), dtype=np.float32
    )

    in_maps = []
    for c in range(NCORES):
        esl = slice(c * EPC, (c + 1) * EPC)
        s_pad = np.full(EPAD, -1, np.int16)
        s_pad[:EPC] = src[esl]
        d_pad = np.full(EPAD, -1, np.int16)
        d_pad[:EPC] = dst[esl]
        nsl = slice(c * NDL, (c + 1) * NDL)
        in_maps.append(
            {
                "table": table,
                "sidx": _wrap_idx(s_pad),
                "didx": _wrap_idx(d_pad),
                "c0a": _arrange_coeff(c0[esl]),
                "c1a": _arrange_coeff(c1[esl]),
                "c2a": _arrange_coeff(c2[esl]),
                "pdl": np.ascontiguousarray(pred[:, nsl].reshape(P, DL_F)),
                "tdl": np.ascontiguousarray(target[:, nsl].reshape(P, DL_F)),
            }
        )

    nc = _build_nc()
    res = run_bass_kernel_spmd(nc, in_maps, list(range(NCORES)))
    LAST_EXEC_NS = res.exec_time_ns
    LAST_PROFILE = res.profile_json

    phy_sum = 0.0
    data_sum = 0.0
    for c in range(NCORES):
        part = np.asarray(res.results[c]["partials"], dtype=np.float64)
        phy_sum += part[:, 0].sum()
        data_sum += part[:, 1].sum()

    data_loss = data_sum / (B * N)
    phy_loss = phy_sum / (B * E)
    total = data_loss + LAMBDA_PHY * phy_loss
    return np.array([total, data_loss, phy_loss], dtype=np.float32)


if __name__ == "__main__":
    rng = np.random.default_rng(0)
    ins = {
        "pred": rng.standard_normal((B, N), dtype=np.float32),
        "target": rng.standard_normal((B, N), dtype=np.float32),
        "prev_target": rng.standard_normal((B, N), dtype=np.float32),
        "c0": rng.random(E, dtype=np.float32),
        "c1": rng.random(E, dtype=np.float32),
        "c2": rng.random(E, dtype=np.float32),
        "edge_index": rng.integers(0, N, (2, E)).astype(np.int64),
    }
    out = kernel(**ins)
    print("kernel out:", out)


# revision 27
# speedup vs baseline: 1.1055x; 1.1055x over previous
import os
import sys

import numpy as np

if "/opt/trn_rl_repo" not in sys.path:
    sys.path.insert(0, "/opt/trn_rl_repo")

import concourse.bass as bass
import concourse.mybir as mybir
import concourse.tile as tile
from concourse import bacc
from concourse.bass_utils import run_bass_kernel_spmd

P = 128
B, N, E = 64, 10000, 320000
LAMBDA_PHY = 0.3
NCORES = 8
EPC = E // NCORES            # 40000 real edges per core
G = (EPC + P - 1) // P       # 313 slot groups per core
EPAD = G * P                 # 40064 (64 pad entries per core)
CHUNK_SLOTS = 8              # 1024 idxs/gather: HW limit is <=1024 per dma_gather
NCHUNKS = (G + CHUNK_SLOTS - 1) // CHUNK_SLOTS  # 39 full chunks + 1-slot tail
NDL = N // NCORES            # 1250 data-loss columns per core
DL_F = B * NDL // P          # 625 free-dim elems for [128, 625] reshape

FP = mybir.dt.float32
I16 = mybir.dt.int16

LAST_EXEC_NS = None
LAST_PROFILE = None

_NC_CACHE = {}


def _build_nc():
    if "nc" in _NC_CACHE:
        return _NC_CACHE["nc"]
    nc = bacc.Bacc(None, target_bir_lowering=False, num_swdge_queues=4)

    table_d = nc.declare_dram_parameter("table", [N, P], FP, isOutput=False)
    sidx_d = nc.declare_dram_parameter("sidx", [P, EPAD // 16], I16, isOutput=False)
    didx_d = nc.declare_dram_parameter("didx", [P, EPAD // 16], I16, isOutput=False)
    c0_d = nc.declare_dram_parameter("c0a", [P, G], FP, isOutput=False)
    c1_d = nc.declare_dram_parameter("c1a", [P, G], FP, isOutput=False)
    c2_d = nc.declare_dram_parameter("c2a", [P, G], FP, isOutput=False)
    pdl_d = nc.declare_dram_parameter("pdl", [P, DL_F], FP, isOutput=False)
    tdl_d = nc.declare_dram_parameter("tdl", [P, DL_F], FP, isOutput=False)
    out_d = nc.declare_dram_parameter("partials", [P, 2], FP, isOutput=True)

    with tile.TileContext(nc) as tc:
        with tc.tile_pool(name="sbuf", bufs=1) as pool:
            sidx_t = pool.tile([P, EPAD // 16], I16)
            didx_t = pool.tile([P, EPAD // 16], I16)
            c0_t = pool.tile([P, G], FP)
            c1_t = pool.tile([P, G], FP)
            c2_t = pool.tile([P, G], FP)
            pdl_t = pool.tile([P, DL_F], FP)
            tdl_t = pool.tile([P, DL_F], FP)
            dd_t = pool.tile([P, DL_F], FP)
            sq_dl = pool.tile([P, DL_F], FP)
            phy_acc = pool.tile([P, 1], FP)
            dacc = pool.tile([P, 1], FP)
            chunk_accs = pool.tile([P, NCHUNKS], FP)

            NBUF = 4
            gs_t = [
                pool.tile([P, CHUNK_SLOTS, P], FP, name=f"gs{i}") for i in range(NBUF)
            ]
            gd_t = [
                pool.tile([P, CHUNK_SLOTS, P], FP, name=f"gd{i}") for i in range(NBUF)
            ]
            a0_t = pool.tile([P, CHUNK_SLOTS, B], FP)
            a1_t = pool.tile([P, CHUNK_SLOTS, B], FP)
            b_t = pool.tile([P, CHUNK_SLOTS, B], FP)
            c_t = pool.tile([P, CHUNK_SLOTS, B], FP)
            r_t = pool.tile([P, CHUNK_SLOTS, B], FP)

            nc.sync.dma_start(out=sidx_t[:], in_=sidx_d[:])
            nc.sync.dma_start(out=didx_t[:], in_=didx_d[:])
            nc.sync.dma_start(out=c0_t[:], in_=c0_d[:])
            nc.sync.dma_start(out=c1_t[:], in_=c1_d[:])
            nc.sync.dma_start(out=c2_t[:], in_=c2_d[:])
            nc.sync.dma_start(out=pdl_t[:], in_=pdl_d[:])
            nc.sync.dma_start(out=tdl_t[:], in_=tdl_d[:])

            # data loss partial: sum((pred - target)^2) over this core's shard
            # (tensor_tensor_reduce crashes the device on this toolchain, so
            # square + separate tensor_reduce instead)
            nc.vector.tensor_tensor(
                out=dd_t[:], in0=pdl_t[:], in1=tdl_t[:], op=mybir.AluOpType.subtract
            )
            nc.vector.tensor_tensor(
                out=sq_dl[:], in0=dd_t[:], in1=dd_t[:], op=mybir.AluOpType.mult
            )
            nc.vector.tensor_reduce(
                out=dacc[:],
                in_=sq_dl[:],
                axis=mybir.AxisListType.X,
                op=mybir.AluOpType.add,
            )

            for k in range(NCHUNKS):
                so = k * CHUNK_SLOTS
                S = min(CHUNK_SLOTS, G - so)
                gs = gs_t[k % NBUF]
                gd = gd_t[k % NBUF]

                n_idx = S * P
                n_real = min(EPC - k * CHUNK_SLOTS * P, n_idx)
                col0 = so * 8  # slot*128/16
                ncol = n_idx // 16

                if n_real < n_idx:
                    # pad entries (negative idxs) are skipped by the
                    # gather; zero their slots so the residual comes out 0
                    pad_p0 = n_real - (S - 1) * P
                    nc.vector.memset(gs[pad_p0:P, S - 1 : S, :], 0.0)
                    nc.vector.memset(gd[pad_p0:P, S - 1 : S, :], 0.0)

                nc.gpsimd.dma_gather(
                    out_ap=gs[:, 0:S, :],
                    in_ap=table_d[:, :],
                    idxs_ap=sidx_t[:, col0 : col0 + ncol],
                    num_idxs=n_idx,
                    num_idxs_reg=n_real,
                    elem_size=P,
                    queue_num=(2 * k) % 4,
                )
                nc.gpsimd.dma_gather(
                    out_ap=gd[:, 0:S, :],
                    in_ap=table_d[:, :],
                    idxs_ap=didx_t[:, col0 : col0 + ncol],
                    num_idxs=n_idx,
                    num_idxs_reg=n_real,
                    elem_size=P,
                    queue_num=(2 * k + 1) % 4,
                )

                c0b = c0_t[:, so : so + S, None].to_broadcast([P, S, B])
                c1b = c1_t[:, so : so + S, None].to_broadcast([P, S, B])
                c2b = c2_t[:, so : so + S, None].to_broadcast([P, S, B])

                mul = mybir.AluOpType.mult
                sub = mybir.AluOpType.subtract
                # a0 = c0 * pred[src], a1 = c1 * prev[src], b = c2 * prev[dst]
                nc.vector.tensor_tensor(
                    out=a0_t[:, 0:S, :], in0=gs[:, 0:S, 0:B], in1=c0b, op=mul
                )
                nc.vector.tensor_tensor(
                    out=a1_t[:, 0:S, :], in0=gs[:, 0:S, B:P], in1=c1b, op=mul
                )
                nc.vector.tensor_tensor(
                    out=b_t[:, 0:S, :], in0=gd[:, 0:S, B:P], in1=c2b, op=mul
                )
                # r = pred[dst] - b - a0 - a1
                nc.vector.tensor_tensor(
                    out=c_t[:, 0:S, :], in0=gd[:, 0:S, 0:B], in1=b_t[:, 0:S, :], op=sub
                )
                nc.vector.tensor_tensor(
                    out=c_t[:, 0:S, :], in0=c_t[:, 0:S, :], in1=a0_t[:, 0:S, :], op=sub
                )
                nc.vector.tensor_tensor(
                    out=r_t[:, 0:S, :], in0=c_t[:, 0:S, :], in1=a1_t[:, 0:S, :], op=sub
                )
                # chunk_accs[:, k] = sum over (S, B) of r^2 per partition
                nc.vector.tensor_tensor(
                    out=b_t[:, 0:S, :], in0=r_t[:, 0:S, :], in1=r_t[:, 0:S, :], op=mul
                )
                nc.vector.tensor_reduce(
                    out=chunk_accs[:, k : k + 1],
                    in_=b_t[:, 0:S, :],
                    axis=mybir.AxisListType.XY,
                    op=mybir.AluOpType.add,
                )

            nc.vector.tensor_reduce(
                out=phy_acc[:],
                in_=chunk_accs[:],
                axis=mybir.AxisListType.X,
                op=mybir.AluOpType.add,
            )
            nc.sync.dma_start(out=out_d[:, 0:1], in_=phy_acc[:])
            nc.sync.dma_start(out=out_d[:, 1:2], in_=dacc[:])

    # Bacc.finalize runs the full lowering pipeline: wait splitting,
    # library loads for DMAGatherAnt, codegen_inst_isa_subclasses
    nc.finalize()
    _NC_CACHE["nc"] = nc
    return nc


def _wrap_idx(idx_pad: np.ndarray) -> np.ndarray:
    # dma_gather layout: index i lives at partition i%16, column i//16,
    # replicated across the 8 groups of 16 partitions
    w16 = idx_pad.reshape(EPAD // 16, 16).T  # [16, EPAD//16]
    return np.ascontiguousarray(np.tile(w16, (8, 1)))  # [128, EPAD//16]


def _arrange_coeff(c_shard: np.ndarray) -> np.ndarray:
    cp = np.zeros(EPAD, np.float32)
    cp[:EPC] = c_shard
    return np.ascontiguousarray(cp.reshape(G, P).T)  # [128, G]


def kernel(**inputs) -> np.ndarray:
    global LAST_EXEC_NS, LAST_PROFILE
    pred = np.ascontiguousarray(np.asarray(inputs["pred"], dtype=np.float32))
    target = np.ascontiguousarray(np.asarray(inputs["target"], dtype=np.float32))
    prev_target = np.ascontiguousarray(
        np.asarray(inputs["prev_target"], dtype=np.float32)
    )
    c0 = np.asarray(inputs["c0"], dtype=np.float32)
    c1 = np.asarray(inputs["c1"], dtype=np.float32)
    c2 = np.asarray(inputs["c2"], dtype=np.float32)
    edge_index = np.asarray(inputs["edge_index"])
    src = edge_index[0].astype(np.int16)
    dst = edge_index[1].astype(np.int16)

    # gather table: row n = [pred[:, n] | prev_target[:, n]]  (512B rows)
    table = np.ascontiguousarray(
        np.concatenate([pred.T, prev_target.T], axis=1), dtype=np.float32
    )

    in_maps = []
    for c in range(NCORES):
        esl = slice(c * EPC, (c + 1) * EPC)
        s_pad = np.full(EPAD, -1, np.int16)
        s_pad[:EPC] = src[esl]
        d_pad = np.full(EPAD, -1, np.int16)
        d_pad[:EPC] = dst[esl]
        nsl = slice(c * NDL, (c + 1) * NDL)
        in_maps.append(
            {
                "table": table,
                "sidx": _wrap_idx(s_pad),
                "didx": _wrap_idx(d_pad),
                "c0a": _arrange_coeff(c0[esl]),
                "c1a": _arrange_coeff(c1[esl]),
                "c2a": _arrange_coeff(c2[esl]),
                "pdl": np.ascontiguousarray(pred[:, nsl].reshape(P, DL_F)),
                "tdl": np.ascontiguousarray(target[:, nsl].reshape(P, DL_F)),
            }
        )

    nc = _build_nc()
    res = run_bass_kernel_spmd(nc, in_maps, list(range(NCORES)))
    LAST_EXEC_NS = res.exec_time_ns
    LAST_PROFILE = res.profile_json

    phy_sum = 0.0
    data_sum = 0.0
    for c in range(NCORES):
        part = np.asarray(res.results[c]["partials"], dtype=np.float64)
        phy_sum += part[:, 0].sum()
        data_sum += part[:, 1].sum()

    data_loss = data_sum / (B * N)
    phy_loss = phy_sum / (B * E)
    total = data_loss + LAMBDA_PHY * phy_loss
    return np.array([total, data_loss, phy_loss], dtype=np.float32)


if __name__ == "__main__":
    rng = np.random.default_rng(0)
    ins = {
        "pred": rng.standard_normal((B, N), dtype=np.float32),
        "target": rng.standard_normal((B, N), dtype=np.float32),
        "prev_target": rng.standard_normal((B, N), dtype=np.float32),
        "c0": rng.random(E, dtype=np.float32),
        "c1": rng.random(E, dtype=np.float32),
        "c2": rng.random(E, dtype=np.float32),
        "edge_index": rng.integers(0, N, (2, E)).astype(np.int64),
    }
    out = kernel(**ins)
    print("kernel out:", out)
